# revision 6
# baseline (speedup 1.0000x reference)
"""Trainium2 Bass kernel v2: 4-layer dense transformer (B=2,T=2048,D=512,H=8,V=32000).

Sharding (DP2 x TP4 over 8 cores): core c handles batch b=c//4, TP rank r=c%4.
The residual stream h is TOKEN-SHARDED: each rank owns a 512-token window
[r*512,(r+1)*512). Per layer:
  LN1(own window) -> AllGather(aw, bf16)              [all tokens visible]
  QKV for the rank's 2 heads over all tokens; attention (causal, windowed)
  -> u [2 heads x 64, 2048] -> AllGather(u)
  -> rank extracts its own 512-token window via a register-driven dynamic
     column slice (offset from a per-core host input)  -> Wo + residual
  LN2(own window) -> FFN(own window) -> residual.
Final LN(own window) -> AllGather -> vocab-shard projection (8000/core).

LN gains/biases are folded into the adjacent weights/biases host-side.
Logits are written bf16 and upcast host-side (plus the lnf_b @ out_w row).
"""
import sys
sys.path.insert(0, "/opt/trn_rl_repo")
import numpy as np
import ml_dtypes

import concourse.bass as bass
import concourse.mybir as mybir
import concourse.tile as tile
from concourse import bacc
from concourse.bass_utils import run_bass_kernel_spmd
from concourse.masks import make_identity

F32 = mybir.dt.float32
F32R = mybir.dt.float32r
BF16 = mybir.dt.bfloat16
AF = mybir.ActivationFunctionType
OP = mybir.AluOpType

N_CORES = 8
GROUPS = [[0, 1, 2, 3], [4, 5, 6, 7]]
P = 128
D = 512            # d_model
T = 2048           # tokens per batch
WT = 512           # tokens per rank window
H_PER = 2          # heads per core
DK = 64
L = 4              # layers
FF = 2048          # d_ff
VSH = 8000         # vocab shard per core
DC = D // P        # 4 D-chunks
TC = T // P        # 16 token chunks
TW = T // WT       # 4 token windows
FC = FF // P       # 16 ff chunks
NV = 500           # vocab tile for the head
EPS = 1e-5
NEG = -1e30


def build_nc_full():
    nc = bacc.Bacc("TRN2", target_bir_lowering=False, debug=False,
                   num_devices=N_CORES)
    h0w = nc.declare_dram_parameter("h0w", [D, WT], F32, isOutput=False)
    wqkT = nc.declare_dram_parameter("wqkT", [L, D, 2 * P], BF16, isOutput=False)
    wvT = nc.declare_dram_parameter("wvT", [L, D, P], BF16, isOutput=False)
    woT = nc.declare_dram_parameter("woT", [L, D, D], BF16, isOutput=False)
    w1T = nc.declare_dram_parameter("w1T", [L, D, FF], BF16, isOutput=False)
    w2T = nc.declare_dram_parameter("w2T", [L, FF, D], BF16, isOutput=False)
    qkbv = nc.declare_dram_parameter("qkbv", [L, P, 2], F32, isOutput=False)
    vbr = nc.declare_dram_parameter("vbr", [L, 1, P], BF16, isOutput=False)
    fb1v = nc.declare_dram_parameter("fb1v", [L, P, FC], F32, isOutput=False)
    fb2v = nc.declare_dram_parameter("fb2v", [L, P, DC], F32, isOutput=False)
    outwT = nc.declare_dram_parameter("outwT", [D, VSH], BF16, isOutput=False)
    woff = nc.declare_dram_parameter("woff", [1, 1], mybir.dt.uint32,
                                     isOutput=False)
    logits = nc.declare_dram_parameter("logits", [T, VSH], BF16, isOutput=True)

    from contextlib import ExitStack
    with tile.TileContext(nc) as tc:
        with ExitStack() as ctx:
            ep = ctx.enter_context
            const = ep(tc.tile_pool(name="const", bufs=1))
            hpool = ep(tc.tile_pool(name="hpool", bufs=1))
            sqp = ep(tc.tile_pool(name="sqp", bufs=2))
            awp = ep(tc.tile_pool(name="awp", bufs=1))
            awfp = ep(tc.tile_pool(name="awfp", bufs=1))
            qkp = ep(tc.tile_pool(name="qkp", bufs=1))
            vxp = ep(tc.tile_pool(name="vxp", bufs=1))
            ptp = ep(tc.tile_pool(name="ptp", bufs=34))
            unp = ep(tc.tile_pool(name="unp", bufs=3))
            utp = ep(tc.tile_pool(name="utp", bufs=1))
            utwp = ep(tc.tile_pool(name="utwp", bufs=1))
            z1p = ep(tc.tile_pool(name="z1p", bufs=17))
            wgt = ep(tc.tile_pool(name="wgt", bufs=1))      # w1/w2
            wgt2 = ep(tc.tile_pool(name="wgt2", bufs=2))    # wqk/wv/wo
            vecs = ep(tc.tile_pool(name="vecs", bufs=2))
            strow = ep(tc.tile_pool(name="strow", bufs=1))
            smallp = ep(tc.tile_pool(name="small", bufs=2))
            owp = ep(tc.tile_pool(name="ow", bufs=2))
            lop = ep(tc.tile_pool(name="lo", bufs=4))
            psS = ep(tc.tile_pool(name="psS", bufs=2, space="PSUM"))
            psQ = ep(tc.tile_pool(name="psQ", bufs=2, space="PSUM"))
            psPV = ep(tc.tile_pool(name="psPV", bufs=2, space="PSUM"))
            pstat = ep(tc.tile_pool(name="pst", bufs=1, space="PSUM"))
            dram = ep(tc.tile_pool(name="dram", bufs=2, space="DRAM"))

            # ---- constants ----
            ident = const.tile([P, P], BF16, tag="ident")
            make_identity(nc, ident)
            # wide causal masks: cmw[j] is [P, WT] with NEG where the column
            # (q = w*WT + f) is behind the row's key token (k = (w*4+j)*P + p),
            # i.e. where f < j*P + p; zero elsewhere. Used as a start=True
            # matmul preload so score matmuls accumulate onto the mask.
            cmw = []
            for j in range(4):
                t = const.tile([P, WT], BF16, tag=f"cmw{j}", name=f"cmw{j}")
                nc.gpsimd.memset(t[:], 0.0)
                nc.gpsimd.affine_select(
                    out=t[:], in_=t[:],
                    compare_op=OP.is_ge, fill=NEG,
                    base=-j * P, pattern=[[1, WT]], channel_multiplier=-1,
                )
                cmw.append(t)
            mean_lhs = const.tile([P, 1], F32, tag="mean_lhs")
            nc.gpsimd.memset(mean_lhs[:], 1.0 / D)
            ones_f = const.tile([1, P], F32, tag="ones_f")
            nc.gpsimd.memset(ones_f[:], 1.0)
            ones_b = const.tile([1, P], BF16, tag="ones_b")
            nc.gpsimd.memset(ones_b[:], 1.0)
            eps_t = const.tile([1, 1], F32, tag="eps_t")
            nc.gpsimd.memset(eps_t[:], EPS)

            # own-window column offset (r*512) as a register for dyn slices
            eng = nc.sync
            woff_reg = eng.alloc_register("woff_reg")
            eng.reg_load(woff_reg, woff[0:1, 0:1])
            woff_v = eng.snap(woff_reg, min_val=0, max_val=T - WT)

            hw = [hpool.tile([P, WT], F32, tag=f"hw{c}", name=f"hw{c}")
                  for c in range(DC)]
            for c in range(DC):
                nc.sync.dma_start(hw[c][:], h0w[c * P:(c + 1) * P, :])

            def load_vec(src, l, w, tag):
                t = vecs.tile([P, w], F32, tag=tag, name=f"{tag}_{l}")
                nc.gpsimd.dma_start(t[:], src[l])
                return t

            def ln_own(out4, pfx, l):
                """LayerNorm (no gain/bias) of hw -> out4 (bf16 [P,WT] x4)."""
                mu_ps = pstat.tile([1, WT], F32, space="PSUM", tag="st",
                                   name=f"{pfx}mu_{l}")
                hw_r = []
                for c in range(DC):
                    hr = sqp.tile([P, WT], F32R, tag="hwr",
                                  name=f"{pfx}hwr{c}_{l}")
                    nc.vector.tensor_copy(hr[:], hw[c][:])
                    hw_r.append(hr)
                    nc.tensor.matmul(mu_ps[:], mean_lhs_r[:], hr[:],
                                     start=(c == 0), stop=(c == DC - 1))
                mu_sb = strow.tile([1, WT], F32, tag="mu_sb",
                                   name=f"{pfx}mu_sb_{l}")
                nc.vector.tensor_copy(mu_sb[:], mu_ps[:])
                ms_ps = pstat.tile([1, WT], F32, space="PSUM", tag="st",
                                   name=f"{pfx}ms_{l}")
                for c in range(DC):
                    sq = sqp.tile([P, WT], F32, tag="sq", name=f"{pfx}sq{c}_{l}")
                    nc.scalar.activation(sq[:], hw[c][:], AF.Square)
                    nc.tensor.matmul(ms_ps[:], mean_lhs[:], sq[:],
                                     start=(c == 0), stop=(c == DC - 1))
                var = strow.tile([1, WT], F32, tag="var", name=f"{pfx}var_{l}")
                nc.vector.tensor_tensor(out=var[:], in0=mu_sb[:], in1=mu_sb[:],
                                        op=OP.mult)
                nc.vector.tensor_tensor(out=var[:], in0=ms_ps[:], in1=var[:],
                                        op=OP.subtract)
                # rstd = exp(-0.5*ln(var+eps)) (stays in the exp/ln table set)
                nc.scalar.activation(var[:], var[:], AF.Ln, bias=eps_t[:])
                nc.scalar.activation(var[:], var[:], AF.Exp, scale=-0.5)
                mu_bc = psQ.tile([P, WT], F32, space="PSUM", tag="mm",
                                 name=f"{pfx}mub_{l}")
                nc.tensor.matmul(mu_bc[:], ones_f[:], mu_sb[:],
                                 start=True, stop=True)
                rs_bc = psQ.tile([P, WT], F32, space="PSUM", tag="mm",
                                 name=f"{pfx}rsb_{l}")
                nc.tensor.matmul(rs_bc[:], ones_f[:], var[:],
                                 start=True, stop=True)
                for c in range(DC):
                    tt = smallp.tile([P, WT], F32, tag="ln_tmp")
                    nc.vector.tensor_tensor(out=tt[:], in0=hw[c][:],
                                            in1=mu_bc[:], op=OP.subtract)
                    nc.vector.tensor_tensor(out=out4[c][:], in0=tt[:],
                                            in1=rs_bc[:], op=OP.mult)

            for l in range(L):
                qkb_t = load_vec(qkbv, l, 2, "qkb")
                fb1 = load_vec(fb1v, l, FC, "fb1")
                fb2 = load_vec(fb2v, l, DC, "fb2")
                vb_t = vecs.tile([1, P], BF16, tag="vb", name=f"vb_{l}")
                nc.gpsimd.dma_start(vb_t[:], vbr[l])
                wqk_sb = [wgt2.tile([P, 2 * P], BF16, tag=f"wqk{k}",
                                    name=f"wqk{k}_{l}") for k in range(DC)]
                wv_sb = [wgt2.tile([P, P], BF16, tag=f"wv{k}",
                                   name=f"wv{k}_{l}") for k in range(DC)]
                wo_sb = [wgt2.tile([P, D], BF16, tag=f"wo{k}",
                                   name=f"wo{k}_{l}") for k in range(DC)]
                w1_sb = [wgt.tile([P, FF], BF16, tag=f"w1{k}",
                                  name=f"w1{k}_{l}") for k in range(DC)]
                w2_sb = [wgt.tile([P, D], BF16, tag=f"w2{k}",
                                  name=f"w2{k}_{l}") for k in range(FC)]
                for k in range(DC):
                    nc.gpsimd.dma_start(wqk_sb[k][:], wqkT[l, k * P:(k + 1) * P, :])
                    nc.gpsimd.dma_start(wv_sb[k][:], wvT[l, k * P:(k + 1) * P, :])
                    nc.gpsimd.dma_start(wo_sb[k][:], woT[l, k * P:(k + 1) * P, :])
                    nc.gpsimd.dma_start(w1_sb[k][:], w1T[l, k * P:(k + 1) * P, :])
                for k in range(FC):
                    nc.gpsimd.dma_start(w2_sb[k][:], w2T[l, k * P:(k + 1) * P, :])

                # ---- LN1 (own window) -> aw; AllGather ----
                aw = [awp.tile([P, WT], BF16, tag=f"aw{c}", name=f"aw{c}_{l}")
                      for c in range(DC)]
                ln_own(aw, "l1", l)
                ag_in = dram.tile([D, WT], BF16, tag="ag_in")
                ag_out = dram.tile([TW * D, WT], BF16, tag="ag_out")
                for c in range(DC):
                    nc.sync.dma_start(ag_in[c * P:(c + 1) * P, :], aw[c][:])
                nc.gpsimd.collective_compute(
                    "AllGather", OP.bypass, replica_groups=GROUPS,
                    ins=[ag_in[:].opt()], outs=[ag_out[:].opt()])

                # ---- QKV over all tokens (rank's 2 heads) ----
                awf = [awfp.tile([P, T], BF16, tag=f"awf{c}", name=f"awf{c}_{l}")
                       for c in range(DC)]
                qk_sb = [qkp.tile([P, T], BF16, tag=f"qk{m}", name=f"qk{m}_{l}")
                         for m in range(2)]
                vx = [[vxp.tile([P, DK + 1], BF16, tag=f"vx{h}_{t}",
                                name=f"vx{h}_{t}_{l}") for t in range(TC)]
                      for h in range(H_PER)]
                if l == 0:
                    for h in range(H_PER):
                        for t in range(TC):
                            nc.vector.memset(vx[h][t][:, DK:DK + 1], 1.0)
                for w in range(TW):
                    wsl = slice(w * WT, (w + 1) * WT)
                    for c in range(DC):
                        nc.sync.dma_start(
                            awf[c][:, wsl],
                            ag_out[w * D + c * P:w * D + (c + 1) * P, :])
                    for m in range(2):
                        pp = psQ.tile([P, WT], F32, space="PSUM", tag="mm")
                        for k in range(DC):
                            nc.tensor.matmul(
                                pp[:], wqk_sb[k][:, m * P:(m + 1) * P],
                                awf[k][:, wsl], start=(k == 0),
                                stop=(k == DC - 1))
                        nc.vector.tensor_scalar(
                            out=qk_sb[m][:, wsl], in0=pp[:],
                            scalar1=qkb_t[:, m:m + 1], scalar2=None,
                            op0=OP.add)
                    for t in range(4):
                        ti = w * 4 + t
                        tsl = slice(ti * P, (ti + 1) * P)
                        vp = psPV.tile([P, P], F32, space="PSUM", tag="pv")
                        for k in range(DC):
                            nc.tensor.matmul(vp[:], awf[k][:, tsl], wv_sb[k][:],
                                             start=(k == 0), stop=False)
                        nc.tensor.matmul(vp[:], ones_b[:], vb_t[:],
                                         start=False, stop=True)
                        for h in range(H_PER):
                            nc.vector.tensor_copy(
                                vx[h][ti][:, :DK],
                                vp[:, h * DK:(h + 1) * DK])

                # ---- attention: scores+exp then PV, per window ----
                uT = utp.tile([P, T], BF16, tag="uT", name=f"uT_{l}")
                for w in range(TW):
                    qsl = slice(w * WT, (w + 1) * WT)
                    pts = {}
                    for h in range(H_PER):
                        hs = slice(h * DK, (h + 1) * DK)
                        for kj in range((w + 1) * 4):
                            sp = psS.tile([P, WT], F32, space="PSUM", tag="sc")
                            if kj >= w * 4:
                                j = kj - w * 4
                                nc.tensor.matmul(sp[:], ident[:], cmw[j][:],
                                                 start=True, stop=False)
                                nc.tensor.matmul(
                                    sp[:], qk_sb[1][hs, kj * P:(kj + 1) * P],
                                    qk_sb[0][hs, qsl], start=False, stop=True)
                            else:
                                nc.tensor.matmul(
                                    sp[:], qk_sb[1][hs, kj * P:(kj + 1) * P],
                                    qk_sb[0][hs, qsl], start=True, stop=True)
                            pt = ptp.tile([P, WT], BF16, tag="pt")
                            nc.scalar.activation(pt[:], sp[:], AF.Exp,
                                                 bias=0.0, scale=0.125)
                            pts[(h, kj)] = pt
                    for qc in range(4):
                        qi = w * 4 + qc
                        ops = []
                        for h in range(H_PER):
                            op = psPV.tile([P, DK + 1], F32, space="PSUM",
                                           tag="pv")
                            for kj in range(qi + 1):
                                nc.tensor.matmul(
                                    op[:], pts[(h, kj)][:, qc * P:(qc + 1) * P],
                                    vx[h][kj][:], start=(kj == 0),
                                    stop=(kj == qi))
                            ops.append(op)
                        un = unp.tile([P, P], BF16, tag="un")
                        for h in range(H_PER):
                            rl = smallp.tile([P, 1], F32, tag="rl")
                            nc.vector.reciprocal(rl[:], ops[h][:, DK:DK + 1])
                            nc.vector.tensor_scalar(
                                out=un[:, h * DK:(h + 1) * DK],
                                in0=ops[h][:, :DK], scalar1=rl[:, :1],
                                scalar2=None, op0=OP.mult)
                        tp = psS.tile([P, P], BF16, space="PSUM", tag="tr", bufs=1)
                        nc.tensor.transpose(out=tp[:], in_=un[:],
                                            identity=ident[:])
                        if qc % 2 == 0:
                            nc.vector.tensor_copy(uT[:, qi * P:(qi + 1) * P],
                                                  tp[:])
                        else:
                            nc.scalar.copy(uT[:, qi * P:(qi + 1) * P], tp[:])

                # ---- AllGather u; extract own window (dyn slice) ----
                agu_in = dram.tile([P, T], BF16, tag="agu_in")
                agu_out = dram.tile([TW * P, T], BF16, tag="agu_out")
                nc.sync.dma_start(agu_in[:], uT[:])
                nc.gpsimd.collective_compute(
                    "AllGather", OP.bypass, replica_groups=GROUPS,
                    ins=[agu_in[:].opt()], outs=[agu_out[:].opt()])
                utw = [utwp.tile([P, WT], BF16, tag=f"utw{k}",
                                 name=f"utw{k}_{l}") for k in range(DC)]
                for k in range(DC):
                    nc.sync.dma_start(
                        utw[k][:],
                        agu_out[k * P:(k + 1) * P, bass.ds(woff_v, WT)])

                # ---- Wo (own window) + residual ----
                for m in range(DC):
                    pp = psQ.tile([P, WT], F32, space="PSUM", tag="mm")
                    for k in range(DC):
                        nc.tensor.matmul(
                            pp[:], wo_sb[k][:, m * P:(m + 1) * P], utw[k][:],
                            start=(k == 0), stop=(k == DC - 1))
                    nc.vector.tensor_add(hw[m][:], hw[m][:], pp[:])

                # ---- LN2 + FFN (own window) ----
                a2 = [awp.tile([P, WT], BF16, tag=f"a2_{c}", name=f"a2{c}_{l}")
                      for c in range(DC)]
                ln_own(a2, "l2", l)
                z1 = [z1p.tile([P, WT], BF16, tag="z1", name=f"z1_{l}_{m}")
                      for m in range(FC)]
                for m in range(FC):
                    pp = psQ.tile([P, WT], F32, space="PSUM", tag="mm")
                    for k in range(DC):
                        nc.tensor.matmul(
                            pp[:], w1_sb[k][:, m * P:(m + 1) * P], a2[k][:],
                            start=(k == 0), stop=(k == DC - 1))
                    nc.scalar.activation(z1[m][:], pp[:], AF.Gelu,
                                         bias=fb1[:, m:m + 1])
                for md in range(DC):
                    pp = psQ.tile([P, WT], F32, space="PSUM", tag="mm")
                    for k in range(FC):
                        nc.tensor.matmul(
                            pp[:], w2_sb[k][:, md * P:(md + 1) * P], z1[k][:],
                            start=(k == 0), stop=(k == FC - 1))
                    tt = smallp.tile([P, WT], F32, tag="ffn_out")
                    nc.vector.tensor_scalar(
                        out=tt[:], in0=pp[:], scalar1=fb2[:, md:md + 1],
                        scalar2=None, op0=OP.add)
                    nc.vector.tensor_add(hw[md][:], hw[md][:], tt[:])

            # ---- final LN + AllGather + vocab-shard projection ----
            af = [awp.tile([P, WT], BF16, tag=f"a2_{c}", name=f"af{c}")
                  for c in range(DC)]
            ln_own(af, "lf", L)
            agf_in = dram.tile([D, WT], BF16, tag="ag_in")
            agf_out = dram.tile([TW * D, WT], BF16, tag="ag_out")
            for c in range(DC):
                nc.sync.dma_start(agf_in[c * P:(c + 1) * P, :], af[c][:])
            nc.gpsimd.collective_compute(
                "AllGather", OP.bypass, replica_groups=GROUPS,
                ins=[agf_in[:].opt()], outs=[agf_out[:].opt()])
            aft = [awfp.tile([P, T], BF16, tag=f"awf{c}", name=f"aft{c}")
                   for c in range(DC)]
            for w in range(TW):
                for c in range(DC):
                    nc.sync.dma_start(
                        aft[c][:, w * WT:(w + 1) * WT],
                        agf_out[w * D + c * P:w * D + (c + 1) * P, :])
            for vc in range(VSH // NV):
                ow_sb = [owp.tile([P, NV], BF16, tag=f"ow{k}",
                                  name=f"ow{vc}_{k}") for k in range(DC)]
                for k in range(DC):
                    nc.gpsimd.dma_start(
                        ow_sb[k][:],
                        outwT[k * P:(k + 1) * P, vc * NV:(vc + 1) * NV])
                for tcx in range(TC):
                    pp = psQ.tile([P, WT], F32, space="PSUM", tag="mm")
                    for k in range(DC):
                        nc.tensor.matmul(
                            pp[:, :NV], aft[k][:, tcx * P:(tcx + 1) * P],
                            ow_sb[k][:], start=(k == 0), stop=(k == DC - 1))
                    lo = lop.tile([P, NV], BF16, tag="lo",
                                  name=f"lo{vc}_{tcx}")
                    if tcx % 2 == 0:
                        nc.scalar.copy(lo[:], pp[:, :NV])
                    else:
                        nc.vector.tensor_copy(lo[:], pp[:, :NV])
                    nc.sync.dma_start(
                        logits[tcx * P:(tcx + 1) * P, vc * NV:(vc + 1) * NV],
                        lo[:])
    nc.compile()
    return nc


_NC_CACHE = None


def _get_nc():
    global _NC_CACHE
    if _NC_CACHE is None:
        _NC_CACHE = build_nc_full()
    return _NC_CACHE


def _vec_tile(v, chunks):
    # [chunks*128] -> [128, chunks] with [p, c] = v[c*128+p]
    return np.ascontiguousarray(np.asarray(v, np.float32).reshape(chunks, P).T)


def prepare_in_maps(inputs):
    return _prep(**inputs)


def _prep(x, embed_w, pos_w, ln1_g, ln1_b, Wqkv, Wo, ln2_g, ln2_b,
          W1, b1, W2, b2, lnf_g, lnf_b, out_w):
    bf = ml_dtypes.bfloat16
    x = np.asarray(x)
    embed_w = np.asarray(embed_w, np.float32)
    pos_w = np.asarray(pos_w, np.float32)
    Wqkv = np.asarray(Wqkv, np.float32)
    Wo_ = np.asarray(Wo, np.float32)
    W1_ = np.asarray(W1, np.float32)
    W2_ = np.asarray(W2, np.float32)
    ln1_g = np.asarray(ln1_g, np.float32)
    ln1_b = np.asarray(ln1_b, np.float32)
    ln2_g = np.asarray(ln2_g, np.float32)
    ln2_b = np.asarray(ln2_b, np.float32)
    b1_ = np.asarray(b1, np.float32)
    b2_ = np.asarray(b2, np.float32)
    lnf_g = np.asarray(lnf_g, np.float32)
    lnf_b = np.asarray(lnf_b, np.float32)
    out_w_ = np.asarray(out_w, np.float32)

    # fold LN2 gain into W1 (columns) and bias into b1
    w1g = W1_ * ln2_g[:, None, :]                    # [L, FF, D]
    b1f = b1_ + np.einsum("lfd,ld->lf", W1_, ln2_b)  # [L, FF]
    w1T = np.ascontiguousarray(w1g.transpose(0, 2, 1)).astype(bf)
    w2T = np.ascontiguousarray(W2_.transpose(0, 2, 1)).astype(bf)
    woT = np.ascontiguousarray(Wo_.transpose(0, 2, 1)).astype(bf)

    shared = {
        "fb1v": np.stack([_vec_tile(b1f[l], FC) for l in range(L)]),
        "fb2v": np.stack([_vec_tile(b2_[l], DC) for l in range(L)]),
        "w1T": w1T, "w2T": w2T, "woT": woT,
    }
    in_maps = []
    for c in range(N_CORES):
        b, r = c // 4, c % 4
        h0 = embed_w[x[b]] + pos_w[:T]                  # [T, D]
        h0w = np.ascontiguousarray(
            h0[r * WT:(r + 1) * WT].T).astype(np.float32)  # [D, WT]
        mu0 = h0.mean(axis=1, keepdims=True)
        var0 = ((h0 - mu0) ** 2).mean(axis=1, keepdims=True)
        aw0 = (h0 - mu0) / np.sqrt(var0 + EPS)          # [T, D]
        aw0T_ = np.ascontiguousarray(aw0.T).astype(bf)  # [D, T]
        heads = [2 * r, 2 * r + 1]
        hrows = np.r_[heads[0] * DK:(heads[0] + 1) * DK,
                      heads[1] * DK:(heads[1] + 1) * DK]
        # fold LN1 gain into Wqkv cols, bias into per-out-dim bias
        wq = Wqkv[:, hrows, :] * ln1_g[:, None, :]            # [L,128,D] q rows
        wk = Wqkv[:, D + hrows, :] * ln1_g[:, None, :]
        wv = Wqkv[:, 2 * D + hrows, :] * ln1_g[:, None, :]
        bq = np.einsum("lhd,ld->lh", Wqkv[:, hrows, :], ln1_b)
        bk = np.einsum("lhd,ld->lh", Wqkv[:, D + hrows, :], ln1_b)
        bv = np.einsum("lhd,ld->lh", Wqkv[:, 2 * D + hrows, :], ln1_b)
        wqkT = np.ascontiguousarray(
            np.concatenate([wq, wk], axis=1).transpose(0, 2, 1)).astype(bf)
        wvp = np.zeros((L, D, 2 * (DK + 1)), np.float32)
        wvp[:, :, 0:DK] = wv.transpose(0, 2, 1)[:, :, 0:DK]
        wvp[:, :, DK + 1:2 * DK + 1] = wv.transpose(0, 2, 1)[:, :, DK:2 * DK]
        wvT_ = np.ascontiguousarray(wvp).astype(bf)
        qkbv = np.stack([np.stack([bq[l], bk[l]], axis=1) for l in range(L)])
        vbp = np.zeros((L, 1, 2 * (DK + 1)), np.float32)
        vbp[:, 0, 0:DK] = bv[:, 0:DK]
        vbp[:, 0, DK] = 1.0
        vbp[:, 0, DK + 1:2 * DK + 1] = bv[:, DK:2 * DK]
        vbp[:, 0, 2 * DK + 1] = 1.0
        vbr_ = np.ascontiguousarray(vbp).astype(bf)
        ow = out_w_[r * VSH:(r + 1) * VSH] * lnf_g[None, :]
        outwT_ = np.ascontiguousarray(ow.T).astype(bf)
        m = {"h0w": h0w, "aw0T": aw0T_, "wqkT": wqkT, "wvT": wvT_,
             "qkbv": qkbv,
             "vbr": vbr_, "outwT": outwT_,
             "woff": np.array([[r * WT]], np.uint32)}
        m.update(shared)
        in_maps.append(m)
    return in_maps


def assemble(results, inputs):
    lnf_b = np.asarray(inputs["lnf_b"], np.float32)
    out_w_ = np.asarray(inputs["out_w"], np.float32)
    out = np.empty((2, T, 4 * VSH), np.float32)
    for c in range(N_CORES):
        b, r = c // 4, c % 4
        row = out_w_[r * VSH:(r + 1) * VSH] @ lnf_b     # [VSH]
        out[b, :, r * VSH:(r + 1) * VSH] = (
            np.asarray(results[c]["logits"], np.float32) + row[None, :])
    return out


def kernel(**inputs):
    nc = _get_nc()
    in_maps = prepare_in_maps(inputs)
    res = run_bass_kernel_spmd(nc, in_maps, list(range(N_CORES)))
    return assemble(res.results, inputs)
